# revision 63
# baseline (speedup 1.0000x reference)
"""Trainium2 Bass kernel for nn_Net_49649821942221.

Computes, for a tiny MLP (SQUARE act -> MUL act -> linear head):
  numerical_b    = f(x) @ Wout.T                     [B, 1]
  numerical_bdot = d/deps f(x + eps*xdot) @ Wout.T   [B]   (forward-mode JVP)
  y              = second-layer activations          [B, H2]
  yy             = x (passthrough)

The reference materializes the full per-sample Jacobian [B, H2, N] via
einsums; only its contraction with xdot is needed, so we propagate a
tangent alongside the primal instead (identical math, ~N x less work).

Sharding: pure data parallel, batch split across 8 NeuronCores.
Layout on device: features on partitions, batch on the free dim.
All host-side prep (transposes, per-chunk packing) is numpy.
"""

import numpy as np

B, N, H1, H2 = 16384, 16, 512, 512
NCORES = 8
BS = B // NCORES          # rows per core (2048)
BT = 512                  # batch block on the free dim
NBLK = BS // BT           # 4 blocks per core
P = 128
MC = H1 // P              # 4 feature chunks of 128

TRACE = False
last_exec_time_ns = None
last_profile = None

_nc_cache = {}


def _build_bass():
    import concourse.bass as bass
    import concourse.mybir as mybir
    from concourse import bacc, tile
    from contextlib import ExitStack

    dt = mybir.dt.float32
    # float32r = same fp32 bits through the 4-XBUS replicated datapath:
    # full-rate (1 cycle/row) matmul instead of fp32's 1/4 rate.
    dtr = mybir.dt.float32r
    AF = mybir.ActivationFunctionType

    nc = bacc.Bacc()

    def mm(out, lhsT, rhs, **kw):
        nc.tensor.matmul(out, lhsT, rhs, **kw)

    # ---- DRAM I/O (per core) ----
    # x pack [16, 2*BS], block-interleaved: per block 1024 cols =
    # [x.T block | xdot.T block] so each block is one contiguous 64KB DMA
    xp_d = nc.declare_dram_parameter("xpack", [N, 2 * BS], dtr, isOutput=False)
    # head pack [16, 3072]: cols 0..1023 = [W1a.T | 2*W1a.T], 1024.. =
    # block 0's [x.T | xdot.T] — one DMA covers the first matmul's deps
    hd_d = nc.declare_dram_parameter("headpack", [N, 2 * H1 + 2 * BT], dtr,
                                     isOutput=False)
    # w2: [H1, H2] = W2a.T / W2b.T
    w2a_d = nc.declare_dram_parameter("w2aT", [H1, H2], dtr, isOutput=False)
    w2b_d = nc.declare_dram_parameter("w2bT", [H1, H2], dtr, isOutput=False)
    # small pack [128, 16]: 4 cols each of b1a, b2a, b2b, woutT (chunked)
    sm_d = nc.declare_dram_parameter("smpack", [P, 3 * MC], dt, isOutput=False)
    wo_d = nc.declare_dram_parameter("woutc", [P, MC], dtr, isOutput=False)

    yT_o = nc.declare_dram_parameter("yT_out", [H2, BS], dtr, isOutput=True)
    bbd_o = nc.declare_dram_parameter("bbd_out", [1, 2 * BS], dt, isOutput=True)

    # y output per block: DRAM [512, BT] viewed as [m, p, BT] from SBUF
    # staging [128, MC*BT] (free block m = feature chunk m).
    yT_v = yT_o.rearrange("(m p) bs -> p m bs", m=MC)

    with ExitStack() as ctx:
        tc = ctx.enter_context(tile.TileContext(nc))

        const = ctx.enter_context(tc.tile_pool(name="const", bufs=1))
        xtp = ctx.enter_context(tc.tile_pool(name="xtp", bufs=4))
        # Startup DMAs: spread across the three issuing engines (SP / ACT /
        # SWDGE) so issue latencies overlap, small transfers first, and the
        # 1MB w2 weights split per k-chunk so they don't monopolize the DMA
        # engines ahead of block 0's inputs.
        hd_sb = const.tile([N, 2 * H1 + 2 * BT], dtr, tag="hd")
        nc.sync.dma_start(out=hd_sb[:], in_=hd_d[:])
        w1_sb = hd_sb[:, 0:2 * H1]
        xblks = []
        for _ in range(NBLK):
            xblk = xtp.tile([N, 2 * BT], dtr, tag="xblk")
            xblks.append(xblk)
        sm_sb = const.tile([P, 3 * MC], dt, tag="sm")
        wo_sb = const.tile([P, MC], dtr, tag="wo")
        nc.scalar.dma_start(out=sm_sb[:], in_=sm_d[:])
        nc.scalar.dma_start(out=wo_sb[:], in_=wo_d[:])
        w2a_sb = []
        w2b_sb = []
        for k in range(MC):
            ta = const.tile([P, H2], dtr, tag=f"w2a{k}")
            tb = const.tile([P, H2], dtr, tag=f"w2b{k}")
            nc.sync.dma_start(out=ta[:], in_=w2a_d[k * P:(k + 1) * P, :])
            nc.gpsimd.dma_start(out=tb[:], in_=w2b_d[k * P:(k + 1) * P, :])
            w2a_sb.append(ta)
            w2b_sb.append(tb)
        # later blocks' inputs ride the SWDGE lane behind the w2b chunks so
        # the early DMA-engine bandwidth goes to the weights
        for blk in range(1, NBLK):
            nc.gpsimd.dma_start(
                out=xblks[blk][:],
                in_=xp_d[:, blk * 2 * BT:(blk + 1) * 2 * BT])

        def b1a(m):
            return sm_sb[:, m:m + 1]

        def b2a(m):
            return sm_sb[:, MC + m:MC + m + 1]

        def b2b(m):
            return sm_sb[:, 2 * MC + m:2 * MC + m + 1]

        def wout(m):
            return wo_sb[:, m:m + 1]

        acts = ctx.enter_context(tc.tile_pool(name="acts", bufs=2))
        ystage = ctx.enter_context(tc.tile_pool(name="ystage", bufs=2))
        small = ctx.enter_context(tc.tile_pool(name="small", bufs=1))

        # PSUM: 8 banks. A/B double-buffered (L1 pz / L2 p1 share slots —
        # sequential phases); C/D single (their consumers drain a half-chunk
        # before reuse thanks to the split k-loop); E/F hold the head
        # accumulators for a whole block.
        ps_a = ctx.enter_context(tc.tile_pool(name="ps_a", bufs=2, space="PSUM"))
        ps_b = ctx.enter_context(tc.tile_pool(name="ps_b", bufs=2, space="PSUM"))
        ps_c = ctx.enter_context(tc.tile_pool(name="ps_c", bufs=1, space="PSUM"))
        ps_d = ctx.enter_context(tc.tile_pool(name="ps_d", bufs=1, space="PSUM"))
        ps_e = ctx.enter_context(tc.tile_pool(name="ps_e", bufs=1, space="PSUM"))
        ps_f = ctx.enter_context(tc.tile_pool(name="ps_f", bufs=1, space="PSUM"))

        # staging for b/bd across all blocks: one DMA out at the end
        bbd_sb = small.tile([1, 2 * BS], dt, tag="bbd")
        b_sb = bbd_sb[:, 0:BS]
        bd_sb = bbd_sb[:, BS:2 * BS]

        # sub-blocks (iblk, offset-within-block, width): the last 512-col
        # block is split in two so the end-of-kernel drain chain is half as
        # deep (its elementwise + head ops are [*,256] and finish sooner)
        SUBS = [(b, 0, BT) for b in range(NBLK)]

        def emit_l1(sub):
            # ---- layer 1: z = W1a x + b1a ; y = z^2 ; yd = z * (2 W1a xd)
            iblk, off, w = sub
            xsrc = hd_sb[:, 2 * H1:] if iblk == 0 else xblks[iblk][:]
            xt = xsrc[:, off:off + w]
            xdt = xsrc[:, BT + off:BT + off + w]
            ys = []
            yds = []
            for m in range(MC):
                msl = slice(m * P, (m + 1) * P)
                msl2 = slice(H1 + m * P, H1 + (m + 1) * P)
                pz = ps_a.tile([P, w], dt, tag="A", name=f"pz{m}")
                pzd = ps_b.tile([P, w], dt, tag="B", name=f"pzd{m}")
                mm(pz[:], w1_sb[:, msl], xt, start=True, stop=True)
                mm(pzd[:], w1_sb[:, msl2], xdt, start=True, stop=True)
                z = acts.tile([P, w], dt, tag=f"z{m}", name=f"z{m}")
                y = acts.tile([P, w], dtr, tag=f"y{m}", name=f"y{m}")
                yd = acts.tile([P, w], dtr, tag=f"yd{m}", name=f"yd{m}")
                nc.scalar.activation(z[:], pz[:], AF.Identity, bias=b1a(m))
                nc.scalar.activation(y[:], pz[:], AF.Square, bias=b1a(m))
                nc.vector.tensor_mul(yd[:], z[:], pzd[:])
                ys.append(y)
                yds.append(yd)
            return ys, yds

        l1_next = emit_l1(SUBS[0])
        for isub, (iblk, off, w) in enumerate(SUBS):
            bsl = slice(iblk * BT + off, iblk * BT + off + w)
            ys, yds = l1_next

            # ---- layer 2 + head, per output-feature chunk ----
            pb = ps_e.tile([1, w], dt, tag="E", name="pb")
            pbd = ps_f.tile([1, w], dt, tag="F", name="pbd")
            yt_a = ystage.tile([P, 2 * w], dtr, tag="yta", name="yt_a")
            yt_b = ystage.tile([P, 2 * w], dtr, tag="ytb", name="yt_b")
            for m in range(MC):
                p1 = ps_a.tile([P, w], dt, tag="A", name="p1")
                p2 = ps_c.tile([P, w], dt, tag="C", name="p2")
                p1d = ps_b.tile([P, w], dt, tag="B", name="p1d")
                p2d = ps_d.tile([P, w], dt, tag="D", name="p2d")
                # split k-loops: p2/p2d's first matmul lands a half-chunk
                # after p2's previous consumer started -> no psum ping-pong
                for k in range(MC):
                    st = k == 0
                    sp = k == MC - 1
                    lha = w2a_sb[k][:, m * P:(m + 1) * P]
                    mm(p1[:], lha, ys[k][:], start=st, stop=sp)
                    mm(p1d[:], lha, yds[k][:], start=st, stop=sp)
                for k in range(MC):
                    st = k == 0
                    sp = k == MC - 1
                    lhb = w2b_sb[k][:, m * P:(m + 1) * P]
                    mm(p2[:], lhb, ys[k][:], start=st, stop=sp)
                    mm(p2d[:], lhb, yds[k][:], start=st, stop=sp)
                z1 = acts.tile([P, w], dt, tag="z1", name="z1")
                z2 = acts.tile([P, w], dt, tag="z2", name="z2")
                nc.scalar.activation(z2[:], p2[:], AF.Identity, bias=b2b(m))
                nc.scalar.activation(z1[:], p1[:], AF.Identity, bias=b2a(m))
                yhalf = yt_a if m < 2 else yt_b
                y2 = yhalf[:, (m % 2) * w:(m % 2 + 1) * w]
                t1 = acts.tile([P, w], dtr, tag="t1", name="t1")
                t2 = acts.tile([P, w], dtr, tag="t2", name="t2")
                nc.vector.tensor_mul(t1[:], p1d[:], z2[:])
                nc.vector.tensor_mul(t2[:], p2d[:], z1[:])
                nc.vector.tensor_mul(y2, z1[:], z2[:])
                # head is linear: wout . y2d = wout . t1 + wout . t2, so
                # accumulate both into pbd and skip materializing y2d
                mm(pbd[:], wout(m), t1[:], start=(m == 0), stop=False)
                mm(pbd[:], wout(m), t2[:], start=False, stop=(m == MC - 1))
                mm(pb[:], wout(m), y2, start=(m == 0), stop=(m == MC - 1))
                if m == 1 and isub + 1 < len(SUBS):
                    l1_next = emit_l1(SUBS[isub + 1])

                if m == 1:
                    nc.gpsimd.dma_start(
                        out=yT_v[:, 0:2, bsl],
                        in_=yt_a[:].rearrange("p (m bt) -> p m bt", m=2))
            nc.gpsimd.dma_start(
                out=yT_v[:, 2:4, bsl],
                in_=yt_b[:].rearrange("p (m bt) -> p m bt", m=2))
            nc.scalar.activation(b_sb[:, bsl], pb[:], AF.Copy)
            nc.scalar.activation(bd_sb[:, bsl], pbd[:], AF.Copy)

        nc.sync.dma_start(out=bbd_o[:], in_=bbd_sb[:])

    nc.compile()
    return nc


def kernel(x, xdot, W1a, b1a, W2a, b2a, W2b, b2b, Wout):
    global last_exec_time_ns, last_profile
    import os
    if not TRACE:
        # the axon client here lacks the NTFF profile hook; make sure a
        # stray BASS_TRACE in the environment can't divert into it
        os.environ["BASS_NEVER_TRACE"] = "1"
    from concourse.bass_utils import run_bass_kernel_spmd

    x = np.ascontiguousarray(x, dtype=np.float32)
    xdot = np.ascontiguousarray(xdot, dtype=np.float32)

    if "nc" not in _nc_cache:
        _nc_cache["nc"] = _build_bass()
    nc = _nc_cache["nc"]

    w1pack = np.concatenate(
        [W1a.T, 2.0 * W1a.T], axis=1).astype(np.float32)       # [N, 1024]
    w2aT = np.ascontiguousarray(W2a.T, dtype=np.float32)       # [H1, H2]
    w2bT = np.ascontiguousarray(W2b.T, dtype=np.float32)
    smpack = np.concatenate(
        [b1a.reshape(MC, P).T, b2a.reshape(MC, P).T,
         b2b.reshape(MC, P).T], axis=1).astype(np.float32)     # [P, 12]
    woutc = np.ascontiguousarray(Wout.reshape(MC, P).T, dtype=np.float32)

    in_maps = []
    for c in range(NCORES):
        rsl = slice(c * BS, (c + 1) * BS)
        xc = x[rsl].T.reshape(N, NBLK, BT)
        xdc = xdot[rsl].T.reshape(N, NBLK, BT)
        # [N, blk, 2, BT]: per block [x | xdot]
        xpack = np.stack([xc, xdc], axis=2).reshape(N, 2 * BS)
        in_maps.append({
            "xpack": np.ascontiguousarray(xpack),
            "headpack": np.ascontiguousarray(
                np.concatenate([w1pack, xpack[:, 0:2 * BT]], axis=1)),
            "w2aT": w2aT, "w2bT": w2bT,
            "smpack": smpack, "woutc": woutc,
        })

    res = run_bass_kernel_spmd(nc, in_maps, list(range(NCORES)), trace=TRACE)
    last_exec_time_ns = res.exec_time_ns
    last_profile = res.profile_json
    results = res.results

    y = np.concatenate([np.asarray(r["yT_out"]).T for r in results], axis=0)
    numerical_b = np.concatenate(
        [np.asarray(r["bbd_out"]).reshape(2, BS)[0] for r in results])[:, None]
    numerical_bdot = np.concatenate(
        [np.asarray(r["bbd_out"]).reshape(2, BS)[1] for r in results])
    return (numerical_b.astype(np.float32), numerical_bdot.astype(np.float32),
            y.astype(np.float32), x)


# revision 69
# speedup vs baseline: 1.0252x; 1.0252x over previous
"""Trainium2 Bass kernel for nn_Net_49649821942221.

Computes, for a tiny MLP (SQUARE act -> MUL act -> linear head):
  numerical_b    = f(x) @ Wout.T                     [B, 1]
  numerical_bdot = d/deps f(x + eps*xdot) @ Wout.T   [B]   (forward-mode JVP)
  y              = second-layer activations          [B, H2]
  yy             = x (passthrough)

The reference materializes the full per-sample Jacobian [B, H2, N] via
einsums; only its contraction with xdot is needed, so we propagate a
tangent alongside the primal instead (identical math, ~N x less work).

Sharding: pure data parallel, batch split across 8 NeuronCores.
Layout on device: features on partitions, batch on the free dim.
All host-side prep (transposes, per-chunk packing) is numpy.
"""

import numpy as np

B, N, H1, H2 = 16384, 16, 512, 512
NCORES = 8
BS = B // NCORES          # rows per core (2048)
BT = 512                  # batch block on the free dim
NBLK = BS // BT           # 4 blocks per core
P = 128
MC = H1 // P              # 4 feature chunks of 128

TRACE = False
last_exec_time_ns = None
last_profile = None

_nc_cache = {}


def _build_bass():
    import concourse.bass as bass
    import concourse.mybir as mybir
    from concourse import bacc, tile
    from contextlib import ExitStack

    dt = mybir.dt.float32
    # float32r = same fp32 bits through the 4-XBUS replicated datapath:
    # full-rate (1 cycle/row) matmul instead of fp32's 1/4 rate.
    dtr = mybir.dt.float32r
    AF = mybir.ActivationFunctionType

    nc = bacc.Bacc()

    def mm(out, lhsT, rhs, **kw):
        nc.tensor.matmul(out, lhsT, rhs, **kw)

    # ---- DRAM I/O (per core) ----
    # x pack [16, 2*BS], block-interleaved: per block 1024 cols =
    # [x.T block | xdot.T block] so each block is one contiguous 64KB DMA
    xp_d = nc.declare_dram_parameter("xpack", [N, 2 * BS], dtr, isOutput=False)
    # head pack [16, 3072]: cols 0..1023 = [W1a.T | 2*W1a.T], 1024.. =
    # block 0's [x.T | xdot.T] — one DMA covers the first matmul's deps
    hd_d = nc.declare_dram_parameter("headpack", [N, 2 * H1 + 2 * BT], dtr,
                                     isOutput=False)
    # w2: [H1, H2] = W2a.T / W2b.T
    w2a_d = nc.declare_dram_parameter("w2aT", [H1, H2], dtr, isOutput=False)
    w2b_d = nc.declare_dram_parameter("w2bT", [H1, H2], dtr, isOutput=False)
    # small pack [128, 16]: 4 cols each of b1a, b2a, b2b, woutT (chunked)
    sm_d = nc.declare_dram_parameter("smpack", [P, 3 * MC], dt, isOutput=False)
    wo_d = nc.declare_dram_parameter("woutc", [P, MC], dtr, isOutput=False)

    yT_o = nc.declare_dram_parameter("yT_out", [H2, BS], dtr, isOutput=True)
    bbd_o = nc.declare_dram_parameter("bbd_out", [1, 2 * BS], dt, isOutput=True)

    # y output per block: DRAM [512, BT] viewed as [m, p, BT] from SBUF
    # staging [128, MC*BT] (free block m = feature chunk m).
    yT_v = yT_o.rearrange("(m p) bs -> p m bs", m=MC)

    with ExitStack() as ctx:
        tc = ctx.enter_context(tile.TileContext(nc))

        const = ctx.enter_context(tc.tile_pool(name="const", bufs=1))
        xtp = ctx.enter_context(tc.tile_pool(name="xtp", bufs=4))
        # Startup DMAs: spread across the three issuing engines (SP / ACT /
        # SWDGE) so issue latencies overlap, small transfers first, and the
        # 1MB w2 weights split per k-chunk so they don't monopolize the DMA
        # engines ahead of block 0's inputs.
        hd_sb = const.tile([N, 2 * H1 + 2 * BT], dtr, tag="hd")
        nc.sync.dma_start(out=hd_sb[:], in_=hd_d[:])
        w1_sb = hd_sb[:, 0:2 * H1]
        xblks = []
        for _ in range(NBLK):
            xblk = xtp.tile([N, 2 * BT], dtr, tag="xblk")
            xblks.append(xblk)
        sm_sb = const.tile([P, 3 * MC], dt, tag="sm")
        wo_sb = const.tile([P, MC], dtr, tag="wo")
        nc.scalar.dma_start(out=sm_sb[:], in_=sm_d[:])
        nc.scalar.dma_start(out=wo_sb[:], in_=wo_d[:])
        w2a_sb = []
        w2b_sb = []
        for k in range(MC):
            ta = const.tile([P, H2], dtr, tag=f"w2a{k}")
            tb = const.tile([P, H2], dtr, tag=f"w2b{k}")
            nc.sync.dma_start(out=ta[:], in_=w2a_d[k * P:(k + 1) * P, :])
            nc.gpsimd.dma_start(out=tb[:], in_=w2b_d[k * P:(k + 1) * P, :])
            w2a_sb.append(ta)
            w2b_sb.append(tb)
        # later blocks' inputs ride the SWDGE lane behind the w2b chunks so
        # the early DMA-engine bandwidth goes to the weights
        for blk in range(1, NBLK):
            nc.gpsimd.dma_start(
                out=xblks[blk][:],
                in_=xp_d[:, blk * 2 * BT:(blk + 1) * 2 * BT])

        def b1a(m):
            return sm_sb[:, m:m + 1]

        def b2a(m):
            return sm_sb[:, MC + m:MC + m + 1]

        def b2b(m):
            return sm_sb[:, 2 * MC + m:2 * MC + m + 1]

        def wout(m):
            return wo_sb[:, m:m + 1]

        acts = ctx.enter_context(tc.tile_pool(name="acts", bufs=2))
        ystage = ctx.enter_context(tc.tile_pool(name="ystage", bufs=2))
        small = ctx.enter_context(tc.tile_pool(name="small", bufs=1))

        # PSUM: 8 banks. A/B double-buffered (L1 pz / L2 p1 share slots —
        # sequential phases); C/D single (their consumers drain a half-chunk
        # before reuse thanks to the split k-loop); E/F hold the head
        # accumulators for a whole block.
        ps_a = ctx.enter_context(tc.tile_pool(name="ps_a", bufs=2, space="PSUM"))
        ps_b = ctx.enter_context(tc.tile_pool(name="ps_b", bufs=2, space="PSUM"))
        ps_c = ctx.enter_context(tc.tile_pool(name="ps_c", bufs=1, space="PSUM"))
        ps_d = ctx.enter_context(tc.tile_pool(name="ps_d", bufs=1, space="PSUM"))
        ps_e = ctx.enter_context(tc.tile_pool(name="ps_e", bufs=1, space="PSUM"))
        ps_f = ctx.enter_context(tc.tile_pool(name="ps_f", bufs=1, space="PSUM"))

        # staging for b/bd across all blocks: one DMA out at the end
        bbd_sb = small.tile([1, 2 * BS], dt, tag="bbd")
        b_sb = bbd_sb[:, 0:BS]
        bd_sb = bbd_sb[:, BS:2 * BS]

        # sub-blocks (iblk, offset-within-block, width): the last 512-col
        # block is split in two so the end-of-kernel drain chain is half as
        # deep (its elementwise + head ops are [*,256] and finish sooner)
        SUBS = [(0, 0, BT), (1, 0, BT), (2, 0, BT),
                (3, 0, BT // 2), (3, BT // 2, BT // 2)]

        def emit_l1(sub):
            # ---- layer 1: z = W1a x + b1a ; y = z^2 ; yd = z * (2 W1a xd)
            iblk, off, w = sub
            xsrc = hd_sb[:, 2 * H1:] if iblk == 0 else xblks[iblk][:]
            xt = xsrc[:, off:off + w]
            xdt = xsrc[:, BT + off:BT + off + w]
            ys = []
            yds = []
            for m in range(MC):
                msl = slice(m * P, (m + 1) * P)
                msl2 = slice(H1 + m * P, H1 + (m + 1) * P)
                pz = ps_a.tile([P, w], dt, tag="A", name=f"pz{m}")
                pzd = ps_b.tile([P, w], dt, tag="B", name=f"pzd{m}")
                mm(pz[:], w1_sb[:, msl], xt, start=True, stop=True)
                mm(pzd[:], w1_sb[:, msl2], xdt, start=True, stop=True)
                z = acts.tile([P, w], dt, tag=f"z{m}", name=f"z{m}")
                y = acts.tile([P, w], dtr, tag=f"y{m}", name=f"y{m}")
                yd = acts.tile([P, w], dtr, tag=f"yd{m}", name=f"yd{m}")
                nc.scalar.activation(z[:], pz[:], AF.Identity, bias=b1a(m))
                nc.scalar.activation(y[:], pz[:], AF.Square, bias=b1a(m))
                nc.vector.tensor_mul(yd[:], z[:], pzd[:])
                ys.append(y)
                yds.append(yd)
            return ys, yds

        l1_next = emit_l1(SUBS[0])
        for isub, (iblk, off, w) in enumerate(SUBS):
            bsl = slice(iblk * BT + off, iblk * BT + off + w)
            ys, yds = l1_next

            # ---- layer 2 + head, per output-feature chunk ----
            pb = ps_e.tile([1, w], dt, tag="E", name="pb")
            pbd = ps_f.tile([1, w], dt, tag="F", name="pbd")
            yt_a = ystage.tile([P, 2 * w], dtr, tag="yta", name="yt_a")
            yt_b = ystage.tile([P, 2 * w], dtr, tag="ytb", name="yt_b")
            for m in range(MC):
                p1 = ps_a.tile([P, w], dt, tag="A", name="p1")
                p2 = ps_c.tile([P, w], dt, tag="C", name="p2")
                p1d = ps_b.tile([P, w], dt, tag="B", name="p1d")
                p2d = ps_d.tile([P, w], dt, tag="D", name="p2d")
                # split k-loops: p2/p2d's first matmul lands a half-chunk
                # after p2's previous consumer started -> no psum ping-pong
                for k in range(MC):
                    st = k == 0
                    sp = k == MC - 1
                    lha = w2a_sb[k][:, m * P:(m + 1) * P]
                    mm(p1[:], lha, ys[k][:], start=st, stop=sp)
                    mm(p1d[:], lha, yds[k][:], start=st, stop=sp)
                for k in range(MC):
                    st = k == 0
                    sp = k == MC - 1
                    lhb = w2b_sb[k][:, m * P:(m + 1) * P]
                    mm(p2[:], lhb, ys[k][:], start=st, stop=sp)
                    mm(p2d[:], lhb, yds[k][:], start=st, stop=sp)
                z1 = acts.tile([P, w], dt, tag="z1", name="z1")
                z2 = acts.tile([P, w], dt, tag="z2", name="z2")
                nc.scalar.activation(z2[:], p2[:], AF.Identity, bias=b2b(m))
                nc.scalar.activation(z1[:], p1[:], AF.Identity, bias=b2a(m))
                yhalf = yt_a if m < 2 else yt_b
                y2 = yhalf[:, (m % 2) * w:(m % 2 + 1) * w]
                t1 = acts.tile([P, w], dtr, tag="t1", name="t1")
                t2 = acts.tile([P, w], dtr, tag="t2", name="t2")
                nc.vector.tensor_mul(t1[:], p1d[:], z2[:])
                nc.vector.tensor_mul(t2[:], p2d[:], z1[:])
                y2d = acts.tile([P, w], dtr, tag="y2d", name="y2d")
                nc.vector.tensor_add(y2d[:], t1[:], t2[:])
                nc.vector.tensor_mul(y2, z1[:], z2[:])
                mm(pbd[:], wout(m), y2d[:], start=(m == 0), stop=(m == MC - 1))
                mm(pb[:], wout(m), y2, start=(m == 0), stop=(m == MC - 1))
                if m == 1 and isub + 1 < len(SUBS):
                    l1_next = emit_l1(SUBS[isub + 1])

                if m == 1:
                    nc.gpsimd.dma_start(
                        out=yT_v[:, 0:2, bsl],
                        in_=yt_a[:].rearrange("p (m bt) -> p m bt", m=2))
            nc.gpsimd.dma_start(
                out=yT_v[:, 2:4, bsl],
                in_=yt_b[:].rearrange("p (m bt) -> p m bt", m=2))
            nc.scalar.activation(b_sb[:, bsl], pb[:], AF.Copy)
            nc.scalar.activation(bd_sb[:, bsl], pbd[:], AF.Copy)

        nc.sync.dma_start(out=bbd_o[:], in_=bbd_sb[:])

    nc.compile()
    return nc


def kernel(x, xdot, W1a, b1a, W2a, b2a, W2b, b2b, Wout):
    global last_exec_time_ns, last_profile
    import os
    if not TRACE:
        # the axon client here lacks the NTFF profile hook; make sure a
        # stray BASS_TRACE in the environment can't divert into it
        os.environ["BASS_NEVER_TRACE"] = "1"
    from concourse.bass_utils import run_bass_kernel_spmd

    x = np.ascontiguousarray(x, dtype=np.float32)
    xdot = np.ascontiguousarray(xdot, dtype=np.float32)

    if "nc" not in _nc_cache:
        _nc_cache["nc"] = _build_bass()
    nc = _nc_cache["nc"]

    w1pack = np.concatenate(
        [W1a.T, 2.0 * W1a.T], axis=1).astype(np.float32)       # [N, 1024]
    w2aT = np.ascontiguousarray(W2a.T, dtype=np.float32)       # [H1, H2]
    w2bT = np.ascontiguousarray(W2b.T, dtype=np.float32)
    smpack = np.concatenate(
        [b1a.reshape(MC, P).T, b2a.reshape(MC, P).T,
         b2b.reshape(MC, P).T], axis=1).astype(np.float32)     # [P, 12]
    woutc = np.ascontiguousarray(Wout.reshape(MC, P).T, dtype=np.float32)

    in_maps = []
    for c in range(NCORES):
        rsl = slice(c * BS, (c + 1) * BS)
        xc = x[rsl].T.reshape(N, NBLK, BT)
        xdc = xdot[rsl].T.reshape(N, NBLK, BT)
        # [N, blk, 2, BT]: per block [x | xdot]
        xpack = np.stack([xc, xdc], axis=2).reshape(N, 2 * BS)
        in_maps.append({
            "xpack": np.ascontiguousarray(xpack),
            "headpack": np.ascontiguousarray(
                np.concatenate([w1pack, xpack[:, 0:2 * BT]], axis=1)),
            "w2aT": w2aT, "w2bT": w2bT,
            "smpack": smpack, "woutc": woutc,
        })

    res = run_bass_kernel_spmd(nc, in_maps, list(range(NCORES)), trace=TRACE)
    last_exec_time_ns = res.exec_time_ns
    last_profile = res.profile_json
    results = res.results

    y = np.concatenate([np.asarray(r["yT_out"]).T for r in results], axis=0)
    numerical_b = np.concatenate(
        [np.asarray(r["bbd_out"]).reshape(2, BS)[0] for r in results])[:, None]
    numerical_bdot = np.concatenate(
        [np.asarray(r["bbd_out"]).reshape(2, BS)[1] for r in results])
    return (numerical_b.astype(np.float32), numerical_bdot.astype(np.float32),
            y.astype(np.float32), x)


# revision 71
# speedup vs baseline: 1.0265x; 1.0013x over previous
"""Trainium2 Bass kernel for nn_Net_49649821942221.

Computes, for a tiny MLP (SQUARE act -> MUL act -> linear head):
  numerical_b    = f(x) @ Wout.T                     [B, 1]
  numerical_bdot = d/deps f(x + eps*xdot) @ Wout.T   [B]   (forward-mode JVP)
  y              = second-layer activations          [B, H2]
  yy             = x (passthrough)

The reference materializes the full per-sample Jacobian [B, H2, N] via
einsums; only its contraction with xdot is needed, so we propagate a
tangent alongside the primal instead (identical math, ~N x less work).

Sharding: pure data parallel, batch split across 8 NeuronCores.
Layout on device: features on partitions, batch on the free dim.
All host-side prep (transposes, per-chunk packing) is numpy.
"""

import numpy as np

B, N, H1, H2 = 16384, 16, 512, 512
NCORES = 8
BS = B // NCORES          # rows per core (2048)
BT = 512                  # batch block on the free dim
NBLK = BS // BT           # 4 blocks per core
P = 128
MC = H1 // P              # 4 feature chunks of 128

TRACE = False
last_exec_time_ns = None
last_profile = None

_nc_cache = {}


def _build_bass():
    import concourse.bass as bass
    import concourse.mybir as mybir
    from concourse import bacc, tile
    from contextlib import ExitStack

    dt = mybir.dt.float32
    # float32r = same fp32 bits through the 4-XBUS replicated datapath:
    # full-rate (1 cycle/row) matmul instead of fp32's 1/4 rate.
    dtr = mybir.dt.float32r
    AF = mybir.ActivationFunctionType

    nc = bacc.Bacc()

    def mm(out, lhsT, rhs, **kw):
        nc.tensor.matmul(out, lhsT, rhs, **kw)

    # ---- DRAM I/O (per core) ----
    # x pack [16, 2*BS], block-interleaved: per block 1024 cols =
    # [x.T block | xdot.T block] so each block is one contiguous 64KB DMA
    xp_d = nc.declare_dram_parameter("xpack", [N, 2 * BS], dtr, isOutput=False)
    # head pack [16, 3072]: cols 0..1023 = [W1a.T | 2*W1a.T], 1024.. =
    # block 0's [x.T | xdot.T] — one DMA covers the first matmul's deps
    hd_d = nc.declare_dram_parameter("headpack", [N, 2 * H1 + 2 * BT], dtr,
                                     isOutput=False)
    # w2: [H1, H2] = W2a.T / W2b.T
    w2a_d = nc.declare_dram_parameter("w2aT", [H1, H2], dtr, isOutput=False)
    w2b_d = nc.declare_dram_parameter("w2bT", [H1, H2], dtr, isOutput=False)
    # small pack [128, 16]: 4 cols each of b1a, b2a, b2b, woutT (chunked)
    sm_d = nc.declare_dram_parameter("smpack", [P, 3 * MC], dt, isOutput=False)
    wo_d = nc.declare_dram_parameter("woutc", [P, MC], dtr, isOutput=False)

    yT_o = nc.declare_dram_parameter("yT_out", [H2, BS], dtr, isOutput=True)
    bbd_o = nc.declare_dram_parameter("bbd_out", [1, 2 * BS], dt, isOutput=True)

    # y output per block: DRAM [512, BT] viewed as [m, p, BT] from SBUF
    # staging [128, MC*BT] (free block m = feature chunk m).
    yT_v = yT_o.rearrange("(m p) bs -> p m bs", m=MC)

    with ExitStack() as ctx:
        tc = ctx.enter_context(tile.TileContext(nc))

        const = ctx.enter_context(tc.tile_pool(name="const", bufs=1))
        xtp = ctx.enter_context(tc.tile_pool(name="xtp", bufs=4))
        # Startup DMAs: spread across the three issuing engines (SP / ACT /
        # SWDGE) so issue latencies overlap, small transfers first, and the
        # 1MB w2 weights split per k-chunk so they don't monopolize the DMA
        # engines ahead of block 0's inputs.
        hd_sb = const.tile([N, 2 * H1 + 2 * BT], dtr, tag="hd")
        nc.sync.dma_start(out=hd_sb[:], in_=hd_d[:])
        w1_sb = hd_sb[:, 0:2 * H1]
        xblks = []
        for _ in range(NBLK):
            xblk = xtp.tile([N, 2 * BT], dtr, tag="xblk")
            xblks.append(xblk)
        sm_sb = const.tile([P, 3 * MC], dt, tag="sm")
        wo_sb = const.tile([P, MC], dtr, tag="wo")
        nc.scalar.dma_start(out=sm_sb[:], in_=sm_d[:])
        nc.scalar.dma_start(out=wo_sb[:], in_=wo_d[:])
        w2a_sb = []
        w2b_sb = []
        for k in range(MC):
            ta = const.tile([P, H2], dtr, tag=f"w2a{k}")
            tb = const.tile([P, H2], dtr, tag=f"w2b{k}")
            nc.sync.dma_start(out=ta[:], in_=w2a_d[k * P:(k + 1) * P, :])
            nc.gpsimd.dma_start(out=tb[:], in_=w2b_d[k * P:(k + 1) * P, :])
            w2a_sb.append(ta)
            w2b_sb.append(tb)
        # later blocks' inputs ride the SWDGE lane behind the w2b chunks so
        # the early DMA-engine bandwidth goes to the weights
        for blk in range(1, NBLK):
            nc.gpsimd.dma_start(
                out=xblks[blk][:],
                in_=xp_d[:, blk * 2 * BT:(blk + 1) * 2 * BT])

        def b1a(m):
            return sm_sb[:, m:m + 1]

        def b2a(m):
            return sm_sb[:, MC + m:MC + m + 1]

        def b2b(m):
            return sm_sb[:, 2 * MC + m:2 * MC + m + 1]

        def wout(m):
            return wo_sb[:, m:m + 1]

        acts = ctx.enter_context(tc.tile_pool(name="acts", bufs=3))
        ystage = ctx.enter_context(tc.tile_pool(name="ystage", bufs=3))
        small = ctx.enter_context(tc.tile_pool(name="small", bufs=1))

        # PSUM: 8 banks. A/B double-buffered (L1 pz / L2 p1 share slots —
        # sequential phases); C/D single (their consumers drain a half-chunk
        # before reuse thanks to the split k-loop); E/F hold the head
        # accumulators for a whole block.
        ps_a = ctx.enter_context(tc.tile_pool(name="ps_a", bufs=2, space="PSUM"))
        ps_b = ctx.enter_context(tc.tile_pool(name="ps_b", bufs=2, space="PSUM"))
        ps_c = ctx.enter_context(tc.tile_pool(name="ps_c", bufs=1, space="PSUM"))
        ps_d = ctx.enter_context(tc.tile_pool(name="ps_d", bufs=1, space="PSUM"))
        ps_e = ctx.enter_context(tc.tile_pool(name="ps_e", bufs=1, space="PSUM"))
        ps_f = ctx.enter_context(tc.tile_pool(name="ps_f", bufs=1, space="PSUM"))

        # staging for b/bd across all blocks: one DMA out at the end
        bbd_sb = small.tile([1, 2 * BS], dt, tag="bbd")
        b_sb = bbd_sb[:, 0:BS]
        bd_sb = bbd_sb[:, BS:2 * BS]

        # sub-blocks (iblk, offset-within-block, width): the last 512-col
        # block is split in two so the end-of-kernel drain chain is half as
        # deep (its elementwise + head ops are [*,256] and finish sooner)
        SUBS = [(0, 0, BT), (1, 0, BT), (2, 0, BT),
                (3, 0, BT // 2), (3, BT // 2, BT // 2)]

        def emit_l1(sub):
            # ---- layer 1: z = W1a x + b1a ; y = z^2 ; yd = z * (2 W1a xd)
            iblk, off, w = sub
            xsrc = hd_sb[:, 2 * H1:] if iblk == 0 else xblks[iblk][:]
            xt = xsrc[:, off:off + w]
            xdt = xsrc[:, BT + off:BT + off + w]
            ys = []
            yds = []
            for m in range(MC):
                msl = slice(m * P, (m + 1) * P)
                msl2 = slice(H1 + m * P, H1 + (m + 1) * P)
                pz = ps_a.tile([P, w], dt, tag="A", name=f"pz{m}")
                pzd = ps_b.tile([P, w], dt, tag="B", name=f"pzd{m}")
                mm(pz[:], w1_sb[:, msl], xt, start=True, stop=True)
                mm(pzd[:], w1_sb[:, msl2], xdt, start=True, stop=True)
                z = acts.tile([P, w], dt, tag=f"z{m}", name=f"z{m}")
                y = acts.tile([P, w], dtr, tag=f"y{m}", name=f"y{m}")
                yd = acts.tile([P, w], dtr, tag=f"yd{m}", name=f"yd{m}")
                nc.scalar.activation(z[:], pz[:], AF.Identity, bias=b1a(m))
                nc.scalar.activation(y[:], pz[:], AF.Square, bias=b1a(m))
                nc.vector.tensor_mul(yd[:], z[:], pzd[:])
                ys.append(y)
                yds.append(yd)
            return ys, yds

        l1_next = emit_l1(SUBS[0])
        for isub, (iblk, off, w) in enumerate(SUBS):
            bsl = slice(iblk * BT + off, iblk * BT + off + w)
            ys, yds = l1_next

            # ---- layer 2 + head, per output-feature chunk ----
            pb = ps_e.tile([1, w], dt, tag="E", name="pb")
            pbd = ps_f.tile([1, w], dt, tag="F", name="pbd")
            yt_a = ystage.tile([P, 2 * w], dtr, tag="yta", name="yt_a")
            yt_b = ystage.tile([P, 2 * w], dtr, tag="ytb", name="yt_b")
            for m in range(MC):
                p1 = ps_a.tile([P, w], dt, tag="A", name="p1")
                p2 = ps_c.tile([P, w], dt, tag="C", name="p2")
                p1d = ps_b.tile([P, w], dt, tag="B", name="p1d")
                p2d = ps_d.tile([P, w], dt, tag="D", name="p2d")
                # split k-loops: p2/p2d's first matmul lands a half-chunk
                # after p2's previous consumer started -> no psum ping-pong
                for k in range(MC):
                    st = k == 0
                    sp = k == MC - 1
                    lha = w2a_sb[k][:, m * P:(m + 1) * P]
                    mm(p1[:], lha, ys[k][:], start=st, stop=sp)
                    mm(p1d[:], lha, yds[k][:], start=st, stop=sp)
                for k in range(MC):
                    st = k == 0
                    sp = k == MC - 1
                    lhb = w2b_sb[k][:, m * P:(m + 1) * P]
                    mm(p2[:], lhb, ys[k][:], start=st, stop=sp)
                    mm(p2d[:], lhb, yds[k][:], start=st, stop=sp)
                z1 = acts.tile([P, w], dt, tag="z1", name="z1")
                z2 = acts.tile([P, w], dt, tag="z2", name="z2")
                nc.scalar.activation(z2[:], p2[:], AF.Identity, bias=b2b(m))
                nc.scalar.activation(z1[:], p1[:], AF.Identity, bias=b2a(m))
                yhalf = yt_a if m < 2 else yt_b
                y2 = yhalf[:, (m % 2) * w:(m % 2 + 1) * w]
                t1 = acts.tile([P, w], dtr, tag="t1", name="t1")
                t2 = acts.tile([P, w], dtr, tag="t2", name="t2")
                nc.vector.tensor_mul(t1[:], p1d[:], z2[:])
                nc.vector.tensor_mul(t2[:], p2d[:], z1[:])
                y2d = acts.tile([P, w], dtr, tag="y2d", name="y2d")
                nc.vector.tensor_add(y2d[:], t1[:], t2[:])
                nc.vector.tensor_mul(y2, z1[:], z2[:])
                mm(pbd[:], wout(m), y2d[:], start=(m == 0), stop=(m == MC - 1))
                mm(pb[:], wout(m), y2, start=(m == 0), stop=(m == MC - 1))
                if m == 1 and isub + 1 < len(SUBS):
                    l1_next = emit_l1(SUBS[isub + 1])

                if m == 1:
                    nc.gpsimd.dma_start(
                        out=yT_v[:, 0:2, bsl],
                        in_=yt_a[:].rearrange("p (m bt) -> p m bt", m=2))
            nc.gpsimd.dma_start(
                out=yT_v[:, 2:4, bsl],
                in_=yt_b[:].rearrange("p (m bt) -> p m bt", m=2))
            nc.scalar.activation(b_sb[:, bsl], pb[:], AF.Copy)
            nc.scalar.activation(bd_sb[:, bsl], pbd[:], AF.Copy)

        nc.sync.dma_start(out=bbd_o[:], in_=bbd_sb[:])

    nc.compile()
    return nc


def kernel(x, xdot, W1a, b1a, W2a, b2a, W2b, b2b, Wout):
    global last_exec_time_ns, last_profile
    import os
    if not TRACE:
        # the axon client here lacks the NTFF profile hook; make sure a
        # stray BASS_TRACE in the environment can't divert into it
        os.environ["BASS_NEVER_TRACE"] = "1"
    from concourse.bass_utils import run_bass_kernel_spmd

    x = np.ascontiguousarray(x, dtype=np.float32)
    xdot = np.ascontiguousarray(xdot, dtype=np.float32)

    if "nc" not in _nc_cache:
        _nc_cache["nc"] = _build_bass()
    nc = _nc_cache["nc"]

    w1pack = np.concatenate(
        [W1a.T, 2.0 * W1a.T], axis=1).astype(np.float32)       # [N, 1024]
    w2aT = np.ascontiguousarray(W2a.T, dtype=np.float32)       # [H1, H2]
    w2bT = np.ascontiguousarray(W2b.T, dtype=np.float32)
    smpack = np.concatenate(
        [b1a.reshape(MC, P).T, b2a.reshape(MC, P).T,
         b2b.reshape(MC, P).T], axis=1).astype(np.float32)     # [P, 12]
    woutc = np.ascontiguousarray(Wout.reshape(MC, P).T, dtype=np.float32)

    in_maps = []
    for c in range(NCORES):
        rsl = slice(c * BS, (c + 1) * BS)
        xc = x[rsl].T.reshape(N, NBLK, BT)
        xdc = xdot[rsl].T.reshape(N, NBLK, BT)
        # [N, blk, 2, BT]: per block [x | xdot]
        xpack = np.stack([xc, xdc], axis=2).reshape(N, 2 * BS)
        in_maps.append({
            "xpack": np.ascontiguousarray(xpack),
            "headpack": np.ascontiguousarray(
                np.concatenate([w1pack, xpack[:, 0:2 * BT]], axis=1)),
            "w2aT": w2aT, "w2bT": w2bT,
            "smpack": smpack, "woutc": woutc,
        })

    res = None
    for attempt in range(3):
        try:
            res = run_bass_kernel_spmd(
                nc, in_maps, list(range(NCORES)), trace=TRACE)
            break
        except Exception:
            # transient NRT/device faults have been observed; retry
            if attempt == 2:
                raise
    last_exec_time_ns = res.exec_time_ns
    last_profile = res.profile_json
    results = res.results

    y = np.concatenate([np.asarray(r["yT_out"]).T for r in results], axis=0)
    numerical_b = np.concatenate(
        [np.asarray(r["bbd_out"]).reshape(2, BS)[0] for r in results])[:, None]
    numerical_bdot = np.concatenate(
        [np.asarray(r["bbd_out"]).reshape(2, BS)[1] for r in results])
    return (numerical_b.astype(np.float32), numerical_bdot.astype(np.float32),
            y.astype(np.float32), x)
